# revision 4
# baseline (speedup 1.0000x reference)
"""MoE FFN (shared SwiGLU + 16-expert top-4 routed FFN) on 8 TRN2 NeuronCores.

Strategy (self-contained, shapes hardcoded for the fixed problem config):
  - Router (tiny: [2048,768]@[768,16]) + softmax/top-k + aux loss on host.
  - Shared SwiGLU FFN: tensor-parallel along d_shared_hidden (3072/8=384
    per core); every core reads all 2048 tokens, produces a partial
    [768,2048] down-projection output; host sums the 8 partials.
  - Routed experts: expert-parallel, 2 experts per core. Host gathers each
    expert's tokens (capacity 576, zero-padded), device computes
    silu(x@Wg)*(x@Wu)@Wd for the gathered block, host scales by combine
    weight and scatter-adds.
  - All matmuls run as float32r (full-rate fp32 PE mode), fp32 PSUM
    accumulation. DMAs are ordered by first use; PE is pre-warmed with
    dummy matmuls so the HAM clock gate opens before the real stream.
"""

import numpy as np

# ---- problem constants (fixed) ----
B, T, D = 2, 1024, 768
BT = B * T
H = 384
SH = 3072
E = 16
TOPK = 4
RHO = 0.5
NNULL = int(E * (1 - RHO) / RHO)

NCORES = 8
EPC = E // NCORES            # experts per core
SHS = SH // NCORES           # shared-hidden shard per core
KD = D // 128                # 6 contraction chunks over D
MS = SHS // 128              # 3 tiles over the SH shard / expert hidden H
DM = D // 128                # 6 output tiles over D
NB = 4                       # token blocks for the shared FFN
NTB = BT // NB               # 512 tokens per block
CAP = 576                    # per-expert token capacity
CB = 2                       # capacity blocks
CTB = CAP // CB              # 288 tokens per capacity block
NWARM = 22                   # PE warmup matmuls (N=256, ~4.7us cold)

_COMPILED = None             # cached Bass program
LAST_RESULT = None           # BassKernelResults of the most recent run
TRACE = False                # test harness can flip this for profiling


def _build_nc():
    import concourse.bacc as bacc
    import concourse.mybir as mybir
    import concourse.tile as tile

    f32 = mybir.dt.float32
    f32r = mybir.dt.float32r
    SILU = mybir.ActivationFunctionType.Silu

    nc = bacc.Bacc("TRN2", target_bir_lowering=False, debug=False)

    xs_d = nc.dram_tensor("xs_t", [D, BT], f32r, kind="ExternalInput").ap()
    swg_d = nc.dram_tensor("swg_t", [D, SHS], f32r, kind="ExternalInput").ap()
    swu_d = nc.dram_tensor("swu_t", [D, SHS], f32r, kind="ExternalInput").ap()
    swd_d = nc.dram_tensor("swd_t", [SHS, D], f32r, kind="ExternalInput").ap()
    ewg_d = nc.dram_tensor("ewg", [EPC, D, H], f32r, kind="ExternalInput").ap()
    ewu_d = nc.dram_tensor("ewu", [EPC, D, H], f32r, kind="ExternalInput").ap()
    ewd_d = nc.dram_tensor("ewd", [EPC, H, D], f32r, kind="ExternalInput").ap()
    xg_d = nc.dram_tensor("xg_t", [EPC, D, CAP], f32r, kind="ExternalInput").ap()
    sh_o = nc.dram_tensor("sh_out", [D, BT], f32, kind="ExternalOutput").ap()
    ro_o = nc.dram_tensor("ro_out", [EPC, D, CAP], f32, kind="ExternalOutput").ap()
    wm_o = nc.dram_tensor("warm_out", [128, 256], f32, kind="ExternalOutput").ap()

    with tile.TileContext(nc) as tc, \
            tc.tile_pool(name="wpool", bufs=1) as wp, \
            tc.tile_pool(name="hpool", bufs=1) as hp, \
            tc.tile_pool(name="opool", bufs=1) as op, \
            tc.tile_pool(name="psum", bufs=1, space="PSUM") as pp:

        # ---- PE warmup: dummy matmuls on memset data, no DMA deps ----
        wrm = wp.tile([128, 256], f32, name="wrm", tag="wrm")
        nc.vector.memset(wrm[:], 0.0)
        pw = None
        for i in range(NWARM):
            pw = pp.tile([128, 256], f32, name=f"pw{i}", tag="pd", bufs=2)
            nc.tensor.matmul(pw[:], wrm[:, :128].bitcast(f32r),
                             wrm[:].bitcast(f32r), start=True, stop=True)
        wo = op.tile([128, 256], f32, name="wo", tag="o", bufs=4)
        nc.vector.tensor_copy(wo[:], pw[:])
        nc.gpsimd.dma_start(wm_o[:], wo[:])

        # ---- SBUF tiles (DMAs emitted in first-use order below) ----
        xs = [[wp.tile([128, NTB], f32r, name=f"xs{k}_{n}", tag=f"xs{k}_{n}")
               for n in range(NB)] for k in range(KD)]
        swg = [wp.tile([128, SHS], f32r, name=f"swg{k}", tag=f"swg{k}")
               for k in range(KD)]
        swu = [wp.tile([128, SHS], f32r, name=f"swu{k}", tag=f"swu{k}")
               for k in range(KD)]
        swd = [wp.tile([128, D], f32r, name=f"swd{m}", tag=f"swd{m}")
               for m in range(MS)]
        exg = [[wp.tile([128, H], f32r, name=f"exg{e}_{k}", tag=f"exg{e}_{k}")
                for k in range(KD)] for e in range(EPC)]
        exu = [[wp.tile([128, H], f32r, name=f"exu{e}_{k}", tag=f"exu{e}_{k}")
                for k in range(KD)] for e in range(EPC)]
        exd = [[wp.tile([128, D], f32r, name=f"exd{e}_{m}", tag=f"exd{e}_{m}")
                for m in range(MS)] for e in range(EPC)]
        xg = [[wp.tile([128, CAP], f32r, name=f"xg{e}_{k}", tag=f"xg{e}_{k}")
               for k in range(KD)] for e in range(EPC)]

        # DMA emission in just-in-time order (single sync queue):
        # swg+xs0 -> swu -> swd -> xs1 -> exg0+xg0 -> exu0 -> xs2 -> xs3
        # -> exg1+xg1 -> exu1 -> exd0 -> exd1
        def ld_xs(n):
            for k in range(KD):
                nc.sync.dma_start(xs[k][n][:],
                                  xs_d[128 * k:128 * (k + 1),
                                       NTB * n:NTB * (n + 1)])

        for k in range(KD):
            nc.sync.dma_start(swg[k][:], swg_d[128 * k:128 * (k + 1), :])
            nc.sync.dma_start(xs[k][0][:], xs_d[128 * k:128 * (k + 1), 0:NTB])
        for k in range(KD):
            nc.sync.dma_start(swu[k][:], swu_d[128 * k:128 * (k + 1), :])
        for m in range(MS):
            nc.sync.dma_start(swd[m][:], swd_d[128 * m:128 * (m + 1), :])
        ld_xs(1)
        for k in range(KD):
            nc.sync.dma_start(exg[0][k][:], ewg_d[0, 128 * k:128 * (k + 1), :])
            nc.sync.dma_start(xg[0][k][:], xg_d[0, 128 * k:128 * (k + 1), :])
        for k in range(KD):
            nc.sync.dma_start(exu[0][k][:], ewu_d[0, 128 * k:128 * (k + 1), :])
        ld_xs(2)
        ld_xs(3)
        for k in range(KD):
            nc.sync.dma_start(exg[1][k][:], ewg_d[1, 128 * k:128 * (k + 1), :])
            nc.sync.dma_start(xg[1][k][:], xg_d[1, 128 * k:128 * (k + 1), :])
        for k in range(KD):
            nc.sync.dma_start(exu[1][k][:], ewu_d[1, 128 * k:128 * (k + 1), :])
        for e in range(EPC):
            for m in range(MS):
                nc.sync.dma_start(exd[e][m][:], ewd_d[e, 128 * m:128 * (m + 1), :])

        # ---- pipeline stages ----
        stages = [("sh", n) for n in range(NB)] + \
                 [("ex", (e, nb)) for e in range(EPC) for nb in range(CB)]

        def stage_up(st):
            kind, idx = st
            if kind == "sh":
                n = idx
                ntok = NTB
                rhs = [xs[k][n][:] for k in range(KD)]
                wg, wu = swg, swu
            else:
                e, nb = idx
                ntok = CTB
                rhs = [xg[e][k][:, CTB * nb:CTB * (nb + 1)] for k in range(KD)]
                wg, wu = exg[e], exu[e]
            hs = []
            pgs = []
            for m in range(MS):
                pg = pp.tile([128, NTB], f32, name=f"pg_{kind}{idx}_{m}",
                             tag="pg", bufs=3)
                for k in range(KD):
                    nc.tensor.matmul(pg[:, :ntok],
                                     wg[k][:, 128 * m:128 * (m + 1)], rhs[k],
                                     start=(k == 0), stop=(k == KD - 1))
                pgs.append(pg)
            h1s = []
            for m in range(MS):
                h1 = hp.tile([128, NTB], f32r, name=f"h1_{kind}{idx}_{m}",
                             tag="h1", bufs=3)
                nc.scalar.activation(h1[:, :ntok], pgs[m][:, :ntok], SILU)
                h1s.append(h1)
            for m in range(MS):
                pu = pp.tile([128, NTB], f32, name=f"pu_{kind}{idx}_{m}",
                             tag="pu", bufs=3)
                for k in range(KD):
                    nc.tensor.matmul(pu[:, :ntok],
                                     wu[k][:, 128 * m:128 * (m + 1)], rhs[k],
                                     start=(k == 0), stop=(k == KD - 1))
                h = hp.tile([128, NTB], f32r, name=f"h_{kind}{idx}_{m}",
                            tag="h", bufs=6)
                nc.vector.tensor_mul(h[:, :ntok], h1s[m][:, :ntok],
                                     pu[:, :ntok].bitcast(f32r))
                hs.append(h)
            return hs

        def stage_down(st, hs):
            kind, idx = st
            if kind == "sh":
                ntok = NTB
                wd = swd
            else:
                e, nb = idx
                ntok = CTB
                wd = exd[e]
            for d in range(DM):
                pd = pp.tile([128, NTB], f32, name=f"pd_{kind}{idx}_{d}",
                             tag="pd", bufs=2)
                for m in range(MS):
                    nc.tensor.matmul(pd[:, :ntok],
                                     wd[m][:, 128 * d:128 * (d + 1)],
                                     hs[m][:, :ntok],
                                     start=(m == 0), stop=(m == MS - 1))
                o = op.tile([128, NTB], f32, name=f"o_{kind}{idx}_{d}",
                            tag="o", bufs=4)
                nc.vector.tensor_copy(o[:, :ntok], pd[:, :ntok])
                if kind == "sh":
                    n = idx
                    nc.gpsimd.dma_start(
                        sh_o[128 * d:128 * (d + 1), NTB * n:NTB * (n + 1)],
                        o[:, :ntok])
                else:
                    nc.gpsimd.dma_start(
                        ro_o[idx[0], 128 * d:128 * (d + 1),
                             CTB * idx[1]:CTB * (idx[1] + 1)],
                        o[:, :ntok])

        prev = None
        for st in stages:
            hs = stage_up(st)
            if prev is not None:
                stage_down(*prev)
            prev = (st, hs)
        stage_down(*prev)

    nc.compile()
    return nc


def _get_compiled():
    global _COMPILED
    if _COMPILED is None:
        _COMPILED = _build_nc()
    return _COMPILED


def _softmax64(x):
    m = x.max(axis=-1, keepdims=True)
    e = np.exp((x - m).astype(np.float64))
    return e / e.sum(axis=-1, keepdims=True)


def _route(x2, gate_w, logit_bias, null_logit):
    """Host router. Returns per-token top-k indices/weights and aux loss."""
    lg = (x2 @ gate_w.T + logit_bias[None, :]).astype(np.float32)
    logits = np.concatenate(
        [lg, np.full((BT, NNULL), np.float32(null_logit), np.float32)], axis=1)
    order = np.argsort(-logits, axis=-1, kind="stable")[:, :TOPK]
    probs = _softmax64(logits).astype(np.float32)
    topk_w = np.take_along_axis(probs, order, axis=-1)
    is_null = order >= E
    real_w = np.where(is_null, np.float32(0), topk_w)
    denom = np.maximum(real_w.sum(-1, keepdims=True), np.float32(1e-6))
    w = (real_w / denom).astype(np.float32)

    # aux loss in float64 for accuracy
    P_real = _softmax64(lg).mean(axis=0)
    flat = order.ravel()
    null_flat = flat >= E
    counts = np.bincount(flat[~null_flat], minlength=E).astype(np.float64)
    f_real = counts / max(counts.sum(), 1e-6)
    L_bal = E * np.sum(f_real * P_real)
    null_rate = is_null.mean()
    L_null = (null_rate - RHO) ** 2
    m = logits.max(axis=-1)
    lse = m.astype(np.float64) + np.log(
        np.exp((logits - m[:, None]).astype(np.float64)).sum(axis=-1))
    L_z = np.mean(lse ** 2)
    aux = 0.02 * L_bal + 0.001 * L_z + 0.01 * L_null
    return order, w, is_null, np.float32(aux)


def _silu_np(v):
    return v * (1.0 / (1.0 + np.exp(-v)))


def kernel(x, gate_w, logit_bias, null_logit, shared_gate_w, shared_up_w,
           shared_down_w, W_gate, W_up, W_down):
    global LAST_RESULT
    from concourse import bass_utils

    x = np.asarray(x, np.float32)
    gate_w = np.asarray(gate_w, np.float32)
    logit_bias = np.asarray(logit_bias, np.float32)
    null_logit = np.asarray(null_logit, np.float32)
    shared_gate_w = np.asarray(shared_gate_w, np.float32)
    shared_up_w = np.asarray(shared_up_w, np.float32)
    shared_down_w = np.asarray(shared_down_w, np.float32)
    W_gate = np.asarray(W_gate, np.float32)
    W_up = np.asarray(W_up, np.float32)
    W_down = np.asarray(W_down, np.float32)

    x2 = np.ascontiguousarray(x.reshape(BT, D))
    order, w, is_null, aux = _route(x2, gate_w, logit_bias, null_logit)

    # per-expert gathered tokens + weights
    idx_e = []
    w_e = []
    for e in range(E):
        rows, cols = np.nonzero(order == e)
        idx_e.append(rows)
        w_e.append(w[rows, cols])

    xs_t = np.ascontiguousarray(x2.T)
    in_maps = []
    for c in range(NCORES):
        sl = slice(c * SHS, (c + 1) * SHS)
        es = [EPC * c + j for j in range(EPC)]
        xg_t = np.zeros((EPC, D, CAP), np.float32)
        for j, e in enumerate(es):
            idx = idx_e[e][:CAP]
            xg_t[j, :, :len(idx)] = x2[idx].T
        in_maps.append({
            "xs_t": xs_t,
            "swg_t": np.ascontiguousarray(shared_gate_w[sl, :].T),
            "swu_t": np.ascontiguousarray(shared_up_w[sl, :].T),
            "swd_t": np.ascontiguousarray(shared_down_w[:, sl].T),
            "ewg": np.ascontiguousarray(W_gate[es]),
            "ewu": np.ascontiguousarray(W_up[es]),
            "ewd": np.ascontiguousarray(W_down[es]),
            "xg_t": xg_t,
        })

    nc = _get_compiled()
    res = bass_utils.run_bass_kernel_spmd(
        nc, in_maps, list(range(NCORES)), trace=TRACE)
    LAST_RESULT = res

    # ---- host combine ----
    yT = np.zeros((D, BT), np.float64)
    for c in range(NCORES):
        yT += res.results[c]["sh_out"]
    Y = np.ascontiguousarray(yT.T)
    for c in range(NCORES):
        ro = res.results[c]["ro_out"]  # [EPC, D, CAP]
        for j in range(EPC):
            e = EPC * c + j
            idx = idx_e[e][:CAP]
            if len(idx) == 0:
                continue
            Y[idx] += ro[j, :, :len(idx)].T.astype(np.float64) \
                * w_e[e][:len(idx), None]
            # capacity overflow fallback (never hit for the fixed input)
            if len(idx_e[e]) > CAP:
                rest = idx_e[e][CAP:]
                wr = w_e[e][CAP:]
                h = _silu_np(x2[rest] @ W_gate[e]) * (x2[rest] @ W_up[e])
                Y[rest] += (h @ W_down[e]).astype(np.float64) * wr[:, None]

    y = Y.astype(np.float32).reshape(B, T, D)
    return y, aux


# revision 7
# speedup vs baseline: 1.1667x; 1.1667x over previous
"""MoE FFN (shared SwiGLU + 16-expert top-4 routed FFN) on 8 TRN2 NeuronCores.

Strategy (self-contained, shapes hardcoded for the fixed problem config):
  - Router (tiny: [2048,768]@[768,16]) + softmax/top-k + aux loss on host.
  - Shared SwiGLU FFN: tensor-parallel along d_shared_hidden (3072/8=384
    per core); every core reads all 2048 tokens, produces a partial
    [768,2048] down-projection output; host sums the 8 partials.
  - Routed experts: expert-parallel, 2 experts per core. Host gathers each
    expert's tokens (capacity 576, zero-padded), device computes
    silu(x@Wg)*(x@Wu)@Wd for the gathered block, host scales by combine
    weight and scatter-adds.
  - All matmuls run as float32r (full-rate fp32 PE mode), fp32 PSUM
    accumulation. DMAs are ordered by first use; PE is pre-warmed with
    dummy matmuls so the HAM clock gate opens before the real stream.
"""

import numpy as np

# ---- problem constants (fixed) ----
B, T, D = 2, 1024, 768
BT = B * T
H = 384
SH = 3072
E = 16
TOPK = 4
RHO = 0.5
NNULL = int(E * (1 - RHO) / RHO)

NCORES = 8
EPC = E // NCORES            # experts per core
SHS = SH // NCORES           # shared-hidden shard per core
KD = D // 128                # 6 contraction chunks over D
MS = SHS // 128              # 3 tiles over the SH shard / expert hidden H
DM = D // 128                # 6 output tiles over D
NB = 4                       # token blocks for the shared FFN
NTB = BT // NB               # 512 tokens per block
CAP = 576                    # per-expert token capacity
CB = 2                       # capacity blocks
CTB = CAP // CB              # 288 tokens per capacity block
NWARM = 16                   # PE warmup matmuls (N=256, ~6.8us cold)

_COMPILED = None             # cached Bass program
LAST_RESULT = None           # BassKernelResults of the most recent run
TRACE = False                # test harness can flip this for profiling


def _build_nc():
    import concourse.bacc as bacc
    import concourse.mybir as mybir
    import concourse.tile as tile

    f32 = mybir.dt.float32
    f32r = mybir.dt.float32r
    SILU = mybir.ActivationFunctionType.Silu

    nc = bacc.Bacc("TRN2", target_bir_lowering=False, debug=False)

    xs_d = nc.dram_tensor("xs_t", [D, BT], f32r, kind="ExternalInput").ap()
    swg_d = nc.dram_tensor("swg_t", [D, SHS], f32r, kind="ExternalInput").ap()
    swu_d = nc.dram_tensor("swu_t", [D, SHS], f32r, kind="ExternalInput").ap()
    swd_d = nc.dram_tensor("swd_t", [SHS, D], f32r, kind="ExternalInput").ap()
    ewg_d = nc.dram_tensor("ewg", [EPC, D, H], f32r, kind="ExternalInput").ap()
    ewu_d = nc.dram_tensor("ewu", [EPC, D, H], f32r, kind="ExternalInput").ap()
    ewd_d = nc.dram_tensor("ewd", [EPC, H, D], f32r, kind="ExternalInput").ap()
    xg_d = nc.dram_tensor("xg_t", [EPC, D, CAP], f32r, kind="ExternalInput").ap()
    sh_o = nc.dram_tensor("sh_out", [D, BT], f32, kind="ExternalOutput").ap()
    ro_o = nc.dram_tensor("ro_out", [EPC, D, CAP], f32, kind="ExternalOutput").ap()
    wm_o = nc.dram_tensor("warm_out", [128, 256], f32, kind="ExternalOutput").ap()

    with tile.TileContext(nc) as tc, \
            tc.tile_pool(name="wpool", bufs=1) as wp, \
            tc.tile_pool(name="hpool", bufs=1) as hp, \
            tc.tile_pool(name="opool", bufs=1) as op, \
            tc.tile_pool(name="psum", bufs=1, space="PSUM") as pp:

        # ---- PE warmup: dummy matmuls on uninitialized data, no deps ----
        wrm = wp.tile([128, 256], f32, name="wrm", tag="wrm")
        nc.gpsimd.memset(wrm[:], 0.0)
        pw = None
        for i in range(NWARM):
            pw = pp.tile([128, 256], f32, name=f"pw{i}", tag="pd", bufs=3)
            nc.tensor.matmul(pw[:], wrm[:, :128].bitcast(f32r),
                             wrm[:].bitcast(f32r), start=True, stop=True)
        wo = op.tile([128, 256], f32, name="wo", tag="o", bufs=4)
        nc.vector.tensor_copy(wo[:], pw[:])
        nc.gpsimd.dma_start(wm_o[:], wo[:])

        # ---- SBUF tiles (DMAs emitted in first-use order below) ----
        xs = [[wp.tile([128, NTB], f32r, name=f"xs{k}_{n}", tag=f"xs{k}_{n}")
               for n in range(NB)] for k in range(KD)]
        swg = [wp.tile([128, SHS], f32r, name=f"swg{k}", tag=f"swg{k}")
               for k in range(KD)]
        swu = [wp.tile([128, SHS], f32r, name=f"swu{k}", tag=f"swu{k}")
               for k in range(KD)]
        swd = [wp.tile([128, D], f32r, name=f"swd{m}", tag=f"swd{m}")
               for m in range(MS)]
        exg = [[wp.tile([128, H], f32r, name=f"exg{e}_{k}", tag=f"exg{e}_{k}")
                for k in range(KD)] for e in range(EPC)]
        exu = [[wp.tile([128, H], f32r, name=f"exu{e}_{k}", tag=f"exu{e}_{k}")
                for k in range(KD)] for e in range(EPC)]
        exd = [[wp.tile([128, D], f32r, name=f"exd{e}_{m}", tag=f"exd{e}_{m}")
                for m in range(MS)] for e in range(EPC)]
        xg = [[wp.tile([128, CAP], f32r, name=f"xg{e}_{k}", tag=f"xg{e}_{k}")
               for k in range(KD)] for e in range(EPC)]

        # DMA emission in just-in-time order (single sync queue):
        # swg+xs0 -> swu -> swd -> xs1 -> exg0+xg0 -> exu0 -> xs2 -> xs3
        # -> exg1+xg1 -> exu1 -> exd0 -> exd1
        def ld_xs(n):
            for k in range(KD):
                nc.sync.dma_start(xs[k][n][:],
                                  xs_d[128 * k:128 * (k + 1),
                                       NTB * n:NTB * (n + 1)])

        for k in range(KD):
            nc.sync.dma_start(swg[k][:], swg_d[128 * k:128 * (k + 1), :])
            nc.sync.dma_start(xs[k][0][:], xs_d[128 * k:128 * (k + 1), 0:NTB])
        for k in range(KD):
            nc.sync.dma_start(swu[k][:], swu_d[128 * k:128 * (k + 1), :])
        for m in range(MS):
            nc.sync.dma_start(swd[m][:], swd_d[128 * m:128 * (m + 1), :])
        ld_xs(1)
        ld_xs(2)
        for k in range(KD):
            nc.sync.dma_start(exg[0][k][:], ewg_d[0, 128 * k:128 * (k + 1), :])
            nc.sync.dma_start(xg[0][k][:], xg_d[0, 128 * k:128 * (k + 1), :])
        for k in range(KD):
            nc.sync.dma_start(exu[0][k][:], ewu_d[0, 128 * k:128 * (k + 1), :])
        ld_xs(3)
        for k in range(KD):
            nc.sync.dma_start(exg[1][k][:], ewg_d[1, 128 * k:128 * (k + 1), :])
            nc.sync.dma_start(xg[1][k][:], xg_d[1, 128 * k:128 * (k + 1), :])
        for k in range(KD):
            nc.sync.dma_start(exu[1][k][:], ewu_d[1, 128 * k:128 * (k + 1), :])
        for e in range(EPC):
            for m in range(MS):
                nc.sync.dma_start(exd[e][m][:], ewd_d[e, 128 * m:128 * (m + 1), :])

        # ---- pipeline stages ----
        stages = [("sh", n) for n in range(NB)] + \
                 [("ex", (e, nb)) for e in range(EPC) for nb in range(CB)]

        def stage_up(st):
            kind, idx = st
            if kind == "sh":
                n = idx
                ntok = NTB
                rhs = [xs[k][n][:] for k in range(KD)]
                wg, wu = swg, swu
            else:
                e, nb = idx
                ntok = CTB
                rhs = [xg[e][k][:, CTB * nb:CTB * (nb + 1)] for k in range(KD)]
                wg, wu = exg[e], exu[e]
            hs = []
            pgs = []
            for m in range(MS):
                pg = pp.tile([128, NTB], f32, name=f"pg_{kind}{idx}_{m}",
                             tag="pg", bufs=2)
                for k in range(KD):
                    nc.tensor.matmul(pg[:, :ntok],
                                     wg[k][:, 128 * m:128 * (m + 1)], rhs[k],
                                     start=(k == 0), stop=(k == KD - 1))
                pgs.append(pg)
            h1s = []
            for m in range(MS):
                h1 = hp.tile([128, NTB], f32r, name=f"h1_{kind}{idx}_{m}",
                             tag="h1", bufs=3)
                nc.scalar.activation(h1[:, :ntok], pgs[m][:, :ntok], SILU)
                h1s.append(h1)
            for m in range(MS):
                pu = pp.tile([128, NTB], f32, name=f"pu_{kind}{idx}_{m}",
                             tag="pu", bufs=2)
                for k in range(KD):
                    nc.tensor.matmul(pu[:, :ntok],
                                     wu[k][:, 128 * m:128 * (m + 1)], rhs[k],
                                     start=(k == 0), stop=(k == KD - 1))
                h = hp.tile([128, NTB], f32r, name=f"h_{kind}{idx}_{m}",
                            tag="h", bufs=6)
                nc.vector.tensor_mul(h[:, :ntok], h1s[m][:, :ntok],
                                     pu[:, :ntok].bitcast(f32r))
                hs.append(h)
            return hs

        def stage_down(st, hs):
            kind, idx = st
            if kind == "sh":
                ntok = NTB
                wd = swd
            else:
                e, nb = idx
                ntok = CTB
                wd = exd[e]
            for d in range(DM):
                pd = pp.tile([128, NTB], f32, name=f"pd_{kind}{idx}_{d}",
                             tag="pd", bufs=3)
                for m in range(MS):
                    nc.tensor.matmul(pd[:, :ntok],
                                     wd[m][:, 128 * d:128 * (d + 1)],
                                     hs[m][:, :ntok],
                                     start=(m == 0), stop=(m == MS - 1))
                o = op.tile([128, NTB], f32, name=f"o_{kind}{idx}_{d}",
                            tag="o", bufs=4)
                nc.vector.tensor_copy(o[:, :ntok], pd[:, :ntok])
                if kind == "sh":
                    n = idx
                    nc.gpsimd.dma_start(
                        sh_o[128 * d:128 * (d + 1), NTB * n:NTB * (n + 1)],
                        o[:, :ntok])
                else:
                    nc.gpsimd.dma_start(
                        ro_o[idx[0], 128 * d:128 * (d + 1),
                             CTB * idx[1]:CTB * (idx[1] + 1)],
                        o[:, :ntok])

        prev = None
        for st in stages:
            hs = stage_up(st)
            if prev is not None:
                stage_down(*prev)
            prev = (st, hs)
        stage_down(*prev)

    nc.compile()
    return nc


def _get_compiled():
    global _COMPILED
    if _COMPILED is None:
        _COMPILED = _build_nc()
    return _COMPILED


def _softmax64(x):
    m = x.max(axis=-1, keepdims=True)
    e = np.exp((x - m).astype(np.float64))
    return e / e.sum(axis=-1, keepdims=True)


def _route(x2, gate_w, logit_bias, null_logit):
    """Host router. Returns per-token top-k indices/weights and aux loss."""
    lg = (x2 @ gate_w.T + logit_bias[None, :]).astype(np.float32)
    logits = np.concatenate(
        [lg, np.full((BT, NNULL), np.float32(null_logit), np.float32)], axis=1)
    order = np.argsort(-logits, axis=-1, kind="stable")[:, :TOPK]
    probs = _softmax64(logits).astype(np.float32)
    topk_w = np.take_along_axis(probs, order, axis=-1)
    is_null = order >= E
    real_w = np.where(is_null, np.float32(0), topk_w)
    denom = np.maximum(real_w.sum(-1, keepdims=True), np.float32(1e-6))
    w = (real_w / denom).astype(np.float32)

    # aux loss in float64 for accuracy
    P_real = _softmax64(lg).mean(axis=0)
    flat = order.ravel()
    null_flat = flat >= E
    counts = np.bincount(flat[~null_flat], minlength=E).astype(np.float64)
    f_real = counts / max(counts.sum(), 1e-6)
    L_bal = E * np.sum(f_real * P_real)
    null_rate = is_null.mean()
    L_null = (null_rate - RHO) ** 2
    m = logits.max(axis=-1)
    lse = m.astype(np.float64) + np.log(
        np.exp((logits - m[:, None]).astype(np.float64)).sum(axis=-1))
    L_z = np.mean(lse ** 2)
    aux = 0.02 * L_bal + 0.001 * L_z + 0.01 * L_null
    return order, w, is_null, np.float32(aux)


def _silu_np(v):
    return v * (1.0 / (1.0 + np.exp(-v)))


def kernel(x, gate_w, logit_bias, null_logit, shared_gate_w, shared_up_w,
           shared_down_w, W_gate, W_up, W_down):
    global LAST_RESULT
    from concourse import bass_utils

    x = np.asarray(x, np.float32)
    gate_w = np.asarray(gate_w, np.float32)
    logit_bias = np.asarray(logit_bias, np.float32)
    null_logit = np.asarray(null_logit, np.float32)
    shared_gate_w = np.asarray(shared_gate_w, np.float32)
    shared_up_w = np.asarray(shared_up_w, np.float32)
    shared_down_w = np.asarray(shared_down_w, np.float32)
    W_gate = np.asarray(W_gate, np.float32)
    W_up = np.asarray(W_up, np.float32)
    W_down = np.asarray(W_down, np.float32)

    x2 = np.ascontiguousarray(x.reshape(BT, D))
    order, w, is_null, aux = _route(x2, gate_w, logit_bias, null_logit)

    # per-expert gathered tokens + weights
    idx_e = []
    w_e = []
    for e in range(E):
        rows, cols = np.nonzero(order == e)
        idx_e.append(rows)
        w_e.append(w[rows, cols])

    xs_t = np.ascontiguousarray(x2.T)
    in_maps = []
    for c in range(NCORES):
        sl = slice(c * SHS, (c + 1) * SHS)
        es = [EPC * c + j for j in range(EPC)]
        xg_t = np.zeros((EPC, D, CAP), np.float32)
        for j, e in enumerate(es):
            idx = idx_e[e][:CAP]
            xg_t[j, :, :len(idx)] = x2[idx].T
        in_maps.append({
            "xs_t": xs_t,
            "swg_t": np.ascontiguousarray(shared_gate_w[sl, :].T),
            "swu_t": np.ascontiguousarray(shared_up_w[sl, :].T),
            "swd_t": np.ascontiguousarray(shared_down_w[:, sl].T),
            "ewg": np.ascontiguousarray(W_gate[es]),
            "ewu": np.ascontiguousarray(W_up[es]),
            "ewd": np.ascontiguousarray(W_down[es]),
            "xg_t": xg_t,
        })

    nc = _get_compiled()
    res = bass_utils.run_bass_kernel_spmd(
        nc, in_maps, list(range(NCORES)), trace=TRACE)
    LAST_RESULT = res

    # ---- host combine ----
    yT = np.zeros((D, BT), np.float64)
    for c in range(NCORES):
        yT += res.results[c]["sh_out"]
    Y = np.ascontiguousarray(yT.T)
    for c in range(NCORES):
        ro = res.results[c]["ro_out"]  # [EPC, D, CAP]
        for j in range(EPC):
            e = EPC * c + j
            idx = idx_e[e][:CAP]
            if len(idx) == 0:
                continue
            Y[idx] += ro[j, :, :len(idx)].T.astype(np.float64) \
                * w_e[e][:len(idx), None]
            # capacity overflow fallback (never hit for the fixed input)
            if len(idx_e[e]) > CAP:
                rest = idx_e[e][CAP:]
                wr = w_e[e][CAP:]
                h = _silu_np(x2[rest] @ W_gate[e]) * (x2[rest] @ W_up[e])
                Y[rest] += (h @ W_down[e]).astype(np.float64) * wr[:, None]

    y = Y.astype(np.float32).reshape(B, T, D)
    return y, aux


# revision 8
# speedup vs baseline: 1.1857x; 1.0163x over previous
"""MoE FFN (shared SwiGLU + 16-expert top-4 routed FFN) on 8 TRN2 NeuronCores.

Strategy (self-contained, shapes hardcoded for the fixed problem config):
  - Router (tiny: [2048,768]@[768,16]) + softmax/top-k + aux loss on host.
  - Shared SwiGLU FFN: tensor-parallel along d_shared_hidden (3072/8=384
    per core); every core reads all 2048 tokens, produces a partial
    [768,2048] down-projection output; host sums the 8 partials.
  - Routed experts: expert-parallel, 2 experts per core. Host gathers each
    expert's tokens (capacity 576, zero-padded), device computes
    silu(x@Wg)*(x@Wu)@Wd for the gathered block, host scales by combine
    weight and scatter-adds.
  - All matmuls run as float32r (full-rate fp32 PE mode), fp32 PSUM
    accumulation. DMAs are ordered by first use; PE is pre-warmed with
    dummy matmuls so the HAM clock gate opens before the real stream.
"""

import numpy as np

# ---- problem constants (fixed) ----
B, T, D = 2, 1024, 768
BT = B * T
H = 384
SH = 3072
E = 16
TOPK = 4
RHO = 0.5
NNULL = int(E * (1 - RHO) / RHO)

NCORES = 8
EPC = E // NCORES            # experts per core
SHS = SH // NCORES           # shared-hidden shard per core
KD = D // 128                # 6 contraction chunks over D
MS = SHS // 128              # 3 tiles over the SH shard / expert hidden H
DM = D // 128                # 6 output tiles over D
NB = 4                       # token blocks for the shared FFN
NTB = BT // NB               # 512 tokens per block
CAP = 576                    # per-expert token capacity
CB = 2                       # capacity blocks
CTB = CAP // CB              # 288 tokens per capacity block
NWARM = 10                   # PE warmup matmuls (N=256, ~4.3us cold)

_COMPILED = None             # cached Bass program
LAST_RESULT = None           # BassKernelResults of the most recent run
TRACE = False                # test harness can flip this for profiling


def _build_nc():
    import concourse.bacc as bacc
    import concourse.mybir as mybir
    import concourse.tile as tile

    f32 = mybir.dt.float32
    f32r = mybir.dt.float32r
    SILU = mybir.ActivationFunctionType.Silu

    nc = bacc.Bacc("TRN2", target_bir_lowering=False, debug=False)

    xs_d = nc.dram_tensor("xs_t", [D, BT], f32r, kind="ExternalInput").ap()
    swg_d = nc.dram_tensor("swg_t", [D, SHS], f32r, kind="ExternalInput").ap()
    swu_d = nc.dram_tensor("swu_t", [D, SHS], f32r, kind="ExternalInput").ap()
    swd_d = nc.dram_tensor("swd_t", [SHS, D], f32r, kind="ExternalInput").ap()
    ewg_d = nc.dram_tensor("ewg", [EPC, D, H], f32r, kind="ExternalInput").ap()
    ewu_d = nc.dram_tensor("ewu", [EPC, D, H], f32r, kind="ExternalInput").ap()
    ewd_d = nc.dram_tensor("ewd", [EPC, H, D], f32r, kind="ExternalInput").ap()
    xg_d = nc.dram_tensor("xg_t", [EPC, D, CAP], f32r, kind="ExternalInput").ap()
    sh_o = nc.dram_tensor("sh_out", [D, BT], f32, kind="ExternalOutput").ap()
    ro_o = nc.dram_tensor("ro_out", [EPC, D, CAP], f32, kind="ExternalOutput").ap()
    wm_o = nc.dram_tensor("warm_out", [128, 256], f32, kind="ExternalOutput").ap()

    with tile.TileContext(nc) as tc, \
            tc.tile_pool(name="wpool", bufs=1) as wp, \
            tc.tile_pool(name="hpool", bufs=1) as hp, \
            tc.tile_pool(name="opool", bufs=1) as op, \
            tc.tile_pool(name="psum", bufs=1, space="PSUM") as pp:

        # ---- PE warmup: dummy matmuls on uninitialized data, no deps ----
        wrm = wp.tile([128, 256], f32, name="wrm", tag="wrm")
        nc.gpsimd.memset(wrm[:], 0.0)
        pw = None
        for i in range(NWARM):
            pw = pp.tile([128, 256], f32, name=f"pw{i}", tag="pd", bufs=3)
            nc.tensor.matmul(pw[:], wrm[:, :128].bitcast(f32r),
                             wrm[:].bitcast(f32r), start=True, stop=True)
        wo = op.tile([128, 256], f32, name="wo", tag="o", bufs=4)
        nc.vector.tensor_copy(wo[:], pw[:])
        nc.gpsimd.dma_start(wm_o[:], wo[:])

        # ---- SBUF tiles (DMAs emitted in first-use order below) ----
        xs = [[wp.tile([128, NTB], f32r, name=f"xs{k}_{n}", tag=f"xs{k}_{n}")
               for n in range(NB)] for k in range(KD)]
        swg = [wp.tile([128, SHS], f32r, name=f"swg{k}", tag=f"swg{k}")
               for k in range(KD)]
        swu = [wp.tile([128, SHS], f32r, name=f"swu{k}", tag=f"swu{k}")
               for k in range(KD)]
        swd = [wp.tile([128, D], f32r, name=f"swd{m}", tag=f"swd{m}")
               for m in range(MS)]
        exg = [[wp.tile([128, H], f32r, name=f"exg{e}_{k}", tag=f"exg{e}_{k}")
                for k in range(KD)] for e in range(EPC)]
        exu = [[wp.tile([128, H], f32r, name=f"exu{e}_{k}", tag=f"exu{e}_{k}")
                for k in range(KD)] for e in range(EPC)]
        exd = [[wp.tile([128, D], f32r, name=f"exd{e}_{m}", tag=f"exd{e}_{m}")
                for m in range(MS)] for e in range(EPC)]
        xg = [[wp.tile([128, CAP], f32r, name=f"xg{e}_{k}", tag=f"xg{e}_{k}")
               for k in range(KD)] for e in range(EPC)]

        # DMA emission in just-in-time order (single sync queue):
        # swg+xs0 -> swu -> swd -> xs1 -> exg0+xg0 -> exu0 -> xs2 -> xs3
        # -> exg1+xg1 -> exu1 -> exd0 -> exd1
        def ld_xs(n):
            for k in range(KD):
                nc.sync.dma_start(xs[k][n][:],
                                  xs_d[128 * k:128 * (k + 1),
                                       NTB * n:NTB * (n + 1)])

        for k in range(KD):
            nc.sync.dma_start(swg[k][:], swg_d[128 * k:128 * (k + 1), :])
            nc.sync.dma_start(xs[k][0][:], xs_d[128 * k:128 * (k + 1), 0:NTB])
        for k in range(KD):
            nc.sync.dma_start(swu[k][:], swu_d[128 * k:128 * (k + 1), :])
        for m in range(MS):
            nc.sync.dma_start(swd[m][:], swd_d[128 * m:128 * (m + 1), :])
        ld_xs(1)
        ld_xs(2)
        for k in range(KD):
            nc.sync.dma_start(exg[0][k][:], ewg_d[0, 128 * k:128 * (k + 1), :])
            nc.sync.dma_start(xg[0][k][:], xg_d[0, 128 * k:128 * (k + 1), :])
        for k in range(KD):
            nc.sync.dma_start(exu[0][k][:], ewu_d[0, 128 * k:128 * (k + 1), :])
        ld_xs(3)
        for k in range(KD):
            nc.sync.dma_start(exg[1][k][:], ewg_d[1, 128 * k:128 * (k + 1), :])
            nc.sync.dma_start(xg[1][k][:], xg_d[1, 128 * k:128 * (k + 1), :])
        for k in range(KD):
            nc.sync.dma_start(exu[1][k][:], ewu_d[1, 128 * k:128 * (k + 1), :])
        for e in range(EPC):
            for m in range(MS):
                nc.sync.dma_start(exd[e][m][:], ewd_d[e, 128 * m:128 * (m + 1), :])

        # ---- pipeline stages ----
        stages = [("sh", n) for n in range(NB)] + \
                 [("ex", (e, nb)) for e in range(EPC) for nb in range(CB)]

        def stage_up(st):
            kind, idx = st
            if kind == "sh":
                n = idx
                ntok = NTB
                rhs = [xs[k][n][:] for k in range(KD)]
                wg, wu = swg, swu
            else:
                e, nb = idx
                ntok = CTB
                rhs = [xg[e][k][:, CTB * nb:CTB * (nb + 1)] for k in range(KD)]
                wg, wu = exg[e], exu[e]
            hs = []
            pgs = []
            for m in range(MS):
                pg = pp.tile([128, NTB], f32, name=f"pg_{kind}{idx}_{m}",
                             tag="pg", bufs=2)
                for k in range(KD):
                    nc.tensor.matmul(pg[:, :ntok],
                                     wg[k][:, 128 * m:128 * (m + 1)], rhs[k],
                                     start=(k == 0), stop=(k == KD - 1))
                pgs.append(pg)
            h1s = []
            for m in range(MS):
                h1 = hp.tile([128, NTB], f32r, name=f"h1_{kind}{idx}_{m}",
                             tag="h1", bufs=3)
                nc.scalar.activation(h1[:, :ntok], pgs[m][:, :ntok], SILU)
                h1s.append(h1)
            for m in range(MS):
                pu = pp.tile([128, NTB], f32, name=f"pu_{kind}{idx}_{m}",
                             tag="pu", bufs=2)
                for k in range(KD):
                    nc.tensor.matmul(pu[:, :ntok],
                                     wu[k][:, 128 * m:128 * (m + 1)], rhs[k],
                                     start=(k == 0), stop=(k == KD - 1))
                h = hp.tile([128, NTB], f32r, name=f"h_{kind}{idx}_{m}",
                            tag="h", bufs=6)
                nc.vector.tensor_mul(h[:, :ntok], h1s[m][:, :ntok],
                                     pu[:, :ntok].bitcast(f32r))
                hs.append(h)
            return hs

        def stage_down(st, hs):
            kind, idx = st
            if kind == "sh":
                ntok = NTB
                wd = swd
            else:
                e, nb = idx
                ntok = CTB
                wd = exd[e]
            for d in range(DM):
                pd = pp.tile([128, NTB], f32, name=f"pd_{kind}{idx}_{d}",
                             tag="pd", bufs=3)
                for m in range(MS):
                    nc.tensor.matmul(pd[:, :ntok],
                                     wd[m][:, 128 * d:128 * (d + 1)],
                                     hs[m][:, :ntok],
                                     start=(m == 0), stop=(m == MS - 1))
                o = op.tile([128, NTB], f32, name=f"o_{kind}{idx}_{d}",
                            tag="o", bufs=4)
                nc.vector.tensor_copy(o[:, :ntok], pd[:, :ntok])
                if kind == "sh":
                    n = idx
                    nc.gpsimd.dma_start(
                        sh_o[128 * d:128 * (d + 1), NTB * n:NTB * (n + 1)],
                        o[:, :ntok])
                else:
                    # sync queue is idle once inputs are done (~70us)
                    nc.sync.dma_start(
                        ro_o[idx[0], 128 * d:128 * (d + 1),
                             CTB * idx[1]:CTB * (idx[1] + 1)],
                        o[:, :ntok])

        prev = None
        for st in stages:
            hs = stage_up(st)
            if prev is not None:
                stage_down(*prev)
            prev = (st, hs)
        stage_down(*prev)

    nc.compile()
    return nc


def _get_compiled():
    global _COMPILED
    if _COMPILED is None:
        _COMPILED = _build_nc()
    return _COMPILED


def _softmax64(x):
    m = x.max(axis=-1, keepdims=True)
    e = np.exp((x - m).astype(np.float64))
    return e / e.sum(axis=-1, keepdims=True)


def _route(x2, gate_w, logit_bias, null_logit):
    """Host router. Returns per-token top-k indices/weights and aux loss."""
    lg = (x2 @ gate_w.T + logit_bias[None, :]).astype(np.float32)
    logits = np.concatenate(
        [lg, np.full((BT, NNULL), np.float32(null_logit), np.float32)], axis=1)
    order = np.argsort(-logits, axis=-1, kind="stable")[:, :TOPK]
    probs = _softmax64(logits).astype(np.float32)
    topk_w = np.take_along_axis(probs, order, axis=-1)
    is_null = order >= E
    real_w = np.where(is_null, np.float32(0), topk_w)
    denom = np.maximum(real_w.sum(-1, keepdims=True), np.float32(1e-6))
    w = (real_w / denom).astype(np.float32)

    # aux loss in float64 for accuracy
    P_real = _softmax64(lg).mean(axis=0)
    flat = order.ravel()
    null_flat = flat >= E
    counts = np.bincount(flat[~null_flat], minlength=E).astype(np.float64)
    f_real = counts / max(counts.sum(), 1e-6)
    L_bal = E * np.sum(f_real * P_real)
    null_rate = is_null.mean()
    L_null = (null_rate - RHO) ** 2
    m = logits.max(axis=-1)
    lse = m.astype(np.float64) + np.log(
        np.exp((logits - m[:, None]).astype(np.float64)).sum(axis=-1))
    L_z = np.mean(lse ** 2)
    aux = 0.02 * L_bal + 0.001 * L_z + 0.01 * L_null
    return order, w, is_null, np.float32(aux)


def _silu_np(v):
    return v * (1.0 / (1.0 + np.exp(-v)))


def kernel(x, gate_w, logit_bias, null_logit, shared_gate_w, shared_up_w,
           shared_down_w, W_gate, W_up, W_down):
    global LAST_RESULT
    from concourse import bass_utils

    x = np.asarray(x, np.float32)
    gate_w = np.asarray(gate_w, np.float32)
    logit_bias = np.asarray(logit_bias, np.float32)
    null_logit = np.asarray(null_logit, np.float32)
    shared_gate_w = np.asarray(shared_gate_w, np.float32)
    shared_up_w = np.asarray(shared_up_w, np.float32)
    shared_down_w = np.asarray(shared_down_w, np.float32)
    W_gate = np.asarray(W_gate, np.float32)
    W_up = np.asarray(W_up, np.float32)
    W_down = np.asarray(W_down, np.float32)

    x2 = np.ascontiguousarray(x.reshape(BT, D))
    order, w, is_null, aux = _route(x2, gate_w, logit_bias, null_logit)

    # per-expert gathered tokens + weights
    idx_e = []
    w_e = []
    for e in range(E):
        rows, cols = np.nonzero(order == e)
        idx_e.append(rows)
        w_e.append(w[rows, cols])

    xs_t = np.ascontiguousarray(x2.T)
    in_maps = []
    for c in range(NCORES):
        sl = slice(c * SHS, (c + 1) * SHS)
        es = [EPC * c + j for j in range(EPC)]
        xg_t = np.zeros((EPC, D, CAP), np.float32)
        for j, e in enumerate(es):
            idx = idx_e[e][:CAP]
            xg_t[j, :, :len(idx)] = x2[idx].T
        in_maps.append({
            "xs_t": xs_t,
            "swg_t": np.ascontiguousarray(shared_gate_w[sl, :].T),
            "swu_t": np.ascontiguousarray(shared_up_w[sl, :].T),
            "swd_t": np.ascontiguousarray(shared_down_w[:, sl].T),
            "ewg": np.ascontiguousarray(W_gate[es]),
            "ewu": np.ascontiguousarray(W_up[es]),
            "ewd": np.ascontiguousarray(W_down[es]),
            "xg_t": xg_t,
        })

    nc = _get_compiled()
    res = bass_utils.run_bass_kernel_spmd(
        nc, in_maps, list(range(NCORES)), trace=TRACE)
    LAST_RESULT = res

    # ---- host combine ----
    yT = np.zeros((D, BT), np.float64)
    for c in range(NCORES):
        yT += res.results[c]["sh_out"]
    Y = np.ascontiguousarray(yT.T)
    for c in range(NCORES):
        ro = res.results[c]["ro_out"]  # [EPC, D, CAP]
        for j in range(EPC):
            e = EPC * c + j
            idx = idx_e[e][:CAP]
            if len(idx) == 0:
                continue
            Y[idx] += ro[j, :, :len(idx)].T.astype(np.float64) \
                * w_e[e][:len(idx), None]
            # capacity overflow fallback (never hit for the fixed input)
            if len(idx_e[e]) > CAP:
                rest = idx_e[e][CAP:]
                wr = w_e[e][CAP:]
                h = _silu_np(x2[rest] @ W_gate[e]) * (x2[rest] @ W_up[e])
                Y[rest] += (h @ W_down[e]).astype(np.float64) * wr[:, None]

    y = Y.astype(np.float32).reshape(B, T, D)
    return y, aux
